# revision 46
# baseline (speedup 1.0000x reference)
"""HandGraphConvNet Trainium2 kernel.

Reference computation (eval-mode 2-layer GCN over a 21-joint hand graph):
    h  = x.reshape(S, B, 21, 2)
    h1 = relu(BN1(adj @ (h @ W1) + b1))      # hidden 21x64 per token
    h2 =      BN2(adj @ (h1 @ W2) + b2)
    out = h2.reshape(S, B, 42) + x

Kernel formulation (per core, pure data parallel over 8 cores, 4096 tokens each):
  - Fold BN scale into weights on the host:
      M1[(j,c),(i,d)] = adj[i,j] * W1[c,d] * k1[d],   k1 = g1*rsqrt(v1+eps)
      bias1[(i,d)]    = ((b1-m1)*k1+be1)[d]           (ones-row in x)
      M2[(j,d),(i,e)] = adj[i,j] * W2[d,e] * k2[e]
      shift2[(i,e)]   = ((b2-m2)*k2+be2)[e]
  - Channels on partitions, tokens on free dim; all matmul operands fp16
    (fp32 matmuls run multi-pass on trn2); PSUM fp32.
  - Tokens are processed in 4 pairs of 512-token chunks.  Chunk A of each
    pair lives in SBUF partitions 0..42 (42 features + ones), chunk B in
    partitions 64..106.  MM1 span s computes hidden k-tile s (128 channels)
    for BOTH chunks concurrently via two PE row-tiles:
      A: lhsT=m1[0:43,s]   rhs=x[0:43]   tile (0,0)   -> psum[:, 0:512]
      B: lhsT=m1[64:107,s] rhs=x[64:107] tile (64,0)  -> psum[:, 512:1024]
    One fused relu evacuation (fp32 psum -> fp16 sbuf, 1024 free elems)
    per span, split across the Scalar and Vector engines (GpSimd cannot
    access PSUM on TRN2).
  - MM2 contracts the 1344 hidden channels (11 k-tiles) + a residual k-tile
    that reads x with [identity; shift2]; the two chunks are PE column
    tiles (0,0)/(0,64) of one accumulation into a single PSUM bank.
  - Every DRAM->SBUF load is a 128-partition DMA instruction so the HWDGE
    splits its descriptors across all 16 SDMA engines (sub-128-partition
    loads serialize on one engine at ~20 GB/s).
  - PE warmup matmuls + ACT table preload run during the input-DMA lead-in;
    MM2 of pair p is interleaved with MM1 of pair p+1 so the PE never
    idles.  Output is stored fp16 (cast back on host) to halve the final
    drain.
"""

import os
import numpy as np

import concourse.bacc as bacc
import concourse.mybir as mybir
from concourse import bass_utils
from concourse.tile import TileContext

F32 = mybir.dt.float32
F16 = mybir.dt.float16

S, B, HD = 256, 128, 42
J, CIN, HIDC = 21, 2, 64
NCORES = 8
SS = S // NCORES              # 32 seq positions per core
NTOK = SS * B                 # 4096 tokens per core
NHID = J * HIDC               # 1344 hidden channels per token
CH = 512                      # token chunk (= one PSUM bank of fp32)
NPAIR = NTOK // (2 * CH)      # 4 chunk pairs
NSPAN = 11                    # hidden k-tiles: 10 full (128) + one 64-wide
KAUG = HD + 1                 # 43 = 42 features + ones row
KT = 11                       # MM2 k-tiles over hidden
NWARM = 5                     # PE warmup matmuls during DMA lead-in
BN_EPS = 1e-5

_CACHE = {}
LAST_RESULT = None            # BassKernelResults of the most recent run (for test.py)


def _build_nc():
    nc = bacc.Bacc()
    x_ext = nc.declare_dram_parameter("x", [128, NPAIR, CH], F16, isOutput=False)
    m1_ext = nc.declare_dram_parameter("m1", [128, NSPAN, 128], F16, isOutput=False)
    m2_ext = nc.declare_dram_parameter("m2", [128, KT + 2, HD], F16, isOutput=False)
    out_ext = nc.declare_dram_parameter("out", [HD, NTOK], F16, isOutput=True)

    relu = mybir.ActivationFunctionType.Relu

    with TileContext(nc) as tc:
        with tc.tile_pool(name="const", bufs=1) as cpool, \
             tc.tile_pool(name="h1p", bufs=2) as h1pool, \
             tc.tile_pool(name="osb", bufs=2) as opool_sb, \
             tc.tile_pool(name="pm", bufs=2, space="PSUM") as mpool, \
             tc.tile_pool(name="po", bufs=2, space="PSUM") as popool:

            # --- input DMAs: every load covers all 128 partitions so its
            # descriptors fan out across the 16 SDMA engines.
            m1_sb = cpool.tile([128, NSPAN, 128], F16)
            nc.scalar.dma_start(out=m1_sb[:, 0:3], in_=m1_ext[:, 0:3])
            nc.scalar.dma_start(out=m1_sb[:, 3:NSPAN], in_=m1_ext[:, 3:NSPAN])
            m2_sb = cpool.tile([128, KT + 2, HD], F16)
            nc.scalar.dma_start(out=m2_sb, in_=m2_ext[:])

            x_sb = cpool.tile([128, NPAIR, CH], F16)
            for p in range(NPAIR):
                nc.sync.dma_start(out=x_sb[:, p], in_=x_ext[:, p])

            # --- PE warmup + ACT table preload, overlapped with input DMA
            wtile = cpool.tile([128, CH], F16)
            nc.vector.memset(wtile, 0.0)
            nc.scalar.activation(wtile[0:1, 0:8], wtile[0:1, 8:16], relu)
            wps = popool.tile([128, CH], F32, tag="po")
            for _ in range(NWARM):
                nc.tensor.matmul(wps, wtile[:, 0:128], wtile, start=True, stop=True)

            # evac engine rotation: GpSimd cannot read PSUM on TRN2, so the
            # relu evacuation splits across Scalar (1.2 GHz) and Vector
            # (0.96 GHz) weighted by their throughput.
            def evac_scalar(dst, src):
                nc.scalar.activation(dst, src, relu)

            def evac_vector(dst, src):
                nc.vector.tensor_scalar_max(dst, src, 0.0)

            # per pair: 21 span-slots (A/B halves of 10 full spans + the
            # packed leftover) fill 7 three-bank psum tiles, each drained by
            # one 1536-wide relu evacuation: 4 on scalar, 3 on vector.
            rotation = [evac_scalar, evac_vector, evac_scalar, evac_vector,
                        evac_scalar, evac_vector, evac_scalar]

            h1_of = {}

            def mm1_group(p, g, h1t):
                # slots q = 3g .. 3g+2; q = 2s+side for s<10, q=20 = leftover
                pt = mpool.tile([128, 3 * CH], F32, tag="pm")
                for q in range(3 * g, 3 * g + 3):
                    off = (q - 3 * g) * CH
                    s, side = q // 2, q % 2
                    if q < 20:
                        if side == 0:
                            nc.tensor.matmul(
                                pt[:, off:off + CH],
                                m1_sb[0:KAUG, s, :],
                                x_sb[0:KAUG, p],
                                start=True, stop=True, tile_position=(0, 0),
                            )
                        else:
                            nc.tensor.matmul(
                                pt[:, off:off + CH],
                                m1_sb[64:64 + KAUG, s, :],
                                x_sb[64:64 + KAUG, p],
                                start=True, stop=True, tile_position=(64, 0),
                            )
                    else:
                        # leftover 64 channels: chunk A -> psum partitions
                        # 0..63, chunk B -> 64..127, in one slot
                        nc.tensor.matmul(
                            pt[0:64, off:off + CH],
                            m1_sb[0:KAUG, 10, 0:64],
                            x_sb[0:KAUG, p],
                            start=True, stop=True, tile_position=(0, 0),
                        )
                        nc.tensor.matmul(
                            pt[64:128, off:off + CH],
                            m1_sb[64:64 + KAUG, 10, 0:64],
                            x_sb[64:64 + KAUG, p],
                            start=True, stop=True, tile_position=(64, 64),
                        )
                rotation[g](h1t[:, g * 3 * CH:(g + 1) * 3 * CH], pt[:, :])

            def mm2_steps(p, po, ts):
                h1t = h1_of[p]
                for t in ts:
                    st, sp = (t == 0), (t == KT)
                    if t < KT - 1:
                        nc.tensor.matmul(
                            po[0:HD, :], m2_sb[:, t, :],
                            h1t[:, 2 * t * CH:(2 * t + 1) * CH],
                            start=st, stop=sp, tile_position=(0, 0),
                        )
                        nc.tensor.matmul(
                            po[64:64 + HD, :], m2_sb[:, t, :],
                            h1t[:, (2 * t + 1) * CH:(2 * t + 2) * CH],
                            start=st, stop=sp, tile_position=(0, 64),
                        )
                    elif t == KT - 1:
                        # leftover hidden k-tile: A at partitions 0..63,
                        # B at 64..127 (weights copy in slot 11 rows 64..127)
                        nc.tensor.matmul(
                            po[0:HD, :], m2_sb[0:64, t, :],
                            h1t[0:64, 20 * CH:21 * CH],
                            start=st, stop=sp, tile_position=(0, 0),
                        )
                        nc.tensor.matmul(
                            po[64:64 + HD, :], m2_sb[64:128, KT, :],
                            h1t[64:128, 20 * CH:21 * CH],
                            start=st, stop=sp, tile_position=(64, 64),
                        )
                    else:
                        # residual + bias shift, reading x directly
                        nc.tensor.matmul(
                            po[0:HD, :], m2_sb[0:KAUG, KT, :],
                            x_sb[0:KAUG, p],
                            start=st, stop=sp, tile_position=(0, 0),
                        )
                        nc.tensor.matmul(
                            po[64:64 + HD, :], m2_sb[64:64 + KAUG, KT + 1, :],
                            x_sb[64:64 + KAUG, p],
                            start=st, stop=sp, tile_position=(64, 64),
                        )

            def mm2_finish(p, po, last=False):
                osb = opool_sb.tile([106, CH], F16)
                if last:
                    # split the copy across both engines and generate the
                    # second DMA on the idle Act HWDGE to shorten the tail
                    nc.scalar.copy(osb[0:HD, :], po[0:HD, :])
                    nc.vector.tensor_copy(osb[64:106, :], po[64:106, :])
                    nc.sync.dma_start(out=out_ext[:, 1024 * p:1024 * p + CH],
                                      in_=osb[0:HD, :])
                    nc.scalar.dma_start(
                        out=out_ext[:, 1024 * p + CH:1024 * (p + 1)],
                        in_=osb[64:64 + HD, :])
                else:
                    nc.vector.tensor_copy(osb, po[0:106, :])
                    nc.sync.dma_start(out=out_ext[:, 1024 * p:1024 * p + CH],
                                      in_=osb[0:HD, :])
                    nc.sync.dma_start(
                        out=out_ext[:, 1024 * p + CH:1024 * (p + 1)],
                        in_=osb[64:64 + HD, :])

            # --- pipeline: MM2 of pair p-1 interleaves with MM1 of pair p
            for p in range(NPAIR):
                h1t = h1pool.tile([128, 21 * CH], F16)
                h1_of[p] = h1t
                if p == 0:
                    for g in range(7):
                        mm1_group(p, g, h1t)
                else:
                    prev = p - 1
                    po = popool.tile([128, CH], F32, tag="po")
                    for g in range(7):
                        mm1_group(p, g, h1t)
                        if g < 6:
                            mm2_steps(prev, po, (2 * g, 2 * g + 1))
                    mm2_finish(prev, po)
                    h1_of.pop(prev)
            # drain: MM2 of the last pair
            p = NPAIR - 1
            po = popool.tile([128, CH], F32, tag="po")
            mm2_steps(p, po, range(KT + 1))
            mm2_finish(p, po, last=True)

    nc.finalize()
    return nc


def _prep_weights(adj, W1, b1, W2, b2, g1, be1, m1, v1, g2, be2, m2, v2):
    adj = np.asarray(adj, np.float64)
    k1 = np.asarray(g1, np.float64) / np.sqrt(np.asarray(v1, np.float64) + BN_EPS)
    k2 = np.asarray(g2, np.float64) / np.sqrt(np.asarray(v2, np.float64) + BN_EPS)
    W1k = np.asarray(W1, np.float64) * k1[None, :]
    W2k = np.asarray(W2, np.float64) * k2[None, :]

    # M1[(j,c), (i,d)] = adj[i,j] * W1[c,d] * k1[d]; row j*2+c, col i*64+d
    M1 = np.einsum('ij,cd->jcid', adj, W1k).reshape(J * CIN, NHID)
    bias1 = np.tile((np.asarray(b1, np.float64) - np.asarray(m1, np.float64)) * k1
                    + np.asarray(be1, np.float64), J)
    M1a = np.concatenate([M1, bias1[None, :]], axis=0)          # (43, 1344)
    M1p = np.zeros((KAUG, NSPAN * 128), np.float32)
    M1p[:, :NHID] = M1a.astype(np.float32)
    m1_packed = np.zeros((128, NSPAN, 128), np.float16)
    for s in range(NSPAN):
        m1_packed[0:KAUG, s, :] = M1p[:, 128 * s:128 * (s + 1)].astype(np.float16)
        m1_packed[64:64 + KAUG, s, :] = m1_packed[0:KAUG, s, :]

    # M2[(j,d), (i,e)] = adj[i,j] * W2[d,e] * k2[e]; row j*64+d, col i*2+e
    M2 = np.einsum('ij,de->jdie', adj, W2k).reshape(NHID, HD)
    shift2 = np.tile((np.asarray(b2, np.float64) - np.asarray(m2, np.float64)) * k2
                     + np.asarray(be2, np.float64), J)
    m2_packed = np.zeros((128, KT + 2, HD), np.float32)
    M2p = np.zeros((KT * 128, HD), np.float32)
    M2p[:NHID] = M2.astype(np.float32)
    for t in range(KT):
        m2_packed[:, t, :] = M2p[t * 128:(t + 1) * 128, :]
    # slot KT (11): rows 0..42 = residual-A (identity + shift2 row), and
    # rows 64..127 = copy of hidden k-tile 10 for chunk B's PE row-tile.
    # slot KT+1 (12): rows 64..106 = residual-B.
    m2_packed[0:HD, KT, :] = np.eye(HD, dtype=np.float32)
    m2_packed[HD, KT, :] = shift2.astype(np.float32)
    m2_packed[64:128, KT, :] = m2_packed[0:64, KT - 1, :]
    m2_packed[64:64 + KAUG, KT + 1, :] = m2_packed[0:KAUG, KT, :]
    return m1_packed, m2_packed.astype(np.float16)


def kernel(x, adj, W1, b1, W2, b2, g1, be1, m1, v1, g2, be2, m2, v2):
    global LAST_RESULT
    x = np.asarray(x, np.float32)
    m1_packed, m2_packed = _prep_weights(adj, W1, b1, W2, b2,
                                         g1, be1, m1, v1, g2, be2, m2, v2)

    if "nc" not in _CACHE:
        _CACHE["nc"] = _build_nc()
    nc = _CACHE["nc"]

    in_maps = []
    for c in range(NCORES):
        xs = np.ascontiguousarray(
            x[c * SS:(c + 1) * SS].reshape(NTOK, HD).T).astype(np.float16)
        # chunk A of pair p -> partitions 0..42, chunk B -> 64..106
        xr = xs.reshape(HD, NPAIR, 2, CH)
        x_pk = np.zeros((128, NPAIR, CH), np.float16)
        x_pk[0:HD] = xr[:, :, 0, :]
        x_pk[HD] = 1.0
        x_pk[64:64 + HD] = xr[:, :, 1, :]
        x_pk[64 + HD] = 1.0
        in_maps.append({
            "x": x_pk,
            "m1": m1_packed,
            "m2": m2_packed,
        })

    trace = bool(int(os.environ.get("KERNEL_TRACE", "0")))
    res = bass_utils.run_bass_kernel_spmd(
        nc, in_maps, list(range(NCORES)), trace=trace,
    )
    LAST_RESULT = res

    out = np.empty((S, B, HD), np.float32)
    for c in range(NCORES):
        oc = res.results[c]["out"]                                # (42, NTOK)
        out[c * SS:(c + 1) * SS] = oc.astype(np.float32).T.reshape(SS, B, HD)
    return out


# revision 48
# speedup vs baseline: 1.0676x; 1.0676x over previous
"""HandGraphConvNet Trainium2 kernel.

Reference computation (eval-mode 2-layer GCN over a 21-joint hand graph):
    h  = x.reshape(S, B, 21, 2)
    h1 = relu(BN1(adj @ (h @ W1) + b1))      # hidden 21x64 per token
    h2 =      BN2(adj @ (h1 @ W2) + b2)
    out = h2.reshape(S, B, 42) + x

Kernel formulation (per core, pure data parallel over 8 cores, 4096 tokens each):
  - Fold BN scale into weights on the host:
      M1[(j,c),(i,d)] = adj[i,j] * W1[c,d] * k1[d],   k1 = g1*rsqrt(v1+eps)
      bias1[(i,d)]    = ((b1-m1)*k1+be1)[d]           (ones-row in x)
      M2[(j,d),(i,e)] = adj[i,j] * W2[d,e] * k2[e]
      shift2[(i,e)]   = ((b2-m2)*k2+be2)[e]
  - Channels on partitions, tokens on free dim; all matmul operands fp16
    (fp32 matmuls run multi-pass on trn2); PSUM fp32.
  - Tokens are processed in 4 pairs of 512-token chunks.  Chunk A of each
    pair lives in SBUF partitions 0..42 (42 features + ones), chunk B in
    partitions 64..106.  MM1 span s computes hidden k-tile s (128 channels)
    for BOTH chunks concurrently via two PE row-tiles:
      A: lhsT=m1[0:43,s]   rhs=x[0:43]   tile (0,0)   -> psum[:, 0:512]
      B: lhsT=m1[64:107,s] rhs=x[64:107] tile (64,0)  -> psum[:, 512:1024]
    One fused relu evacuation (fp32 psum -> fp16 sbuf, 1024 free elems)
    per span, split across the Scalar and Vector engines (GpSimd cannot
    access PSUM on TRN2).
  - MM2 contracts the 1344 hidden channels (11 k-tiles) + a residual k-tile
    that reads x with [identity; shift2]; the two chunks are PE column
    tiles (0,0)/(0,64) of one accumulation into a single PSUM bank.
  - Every DRAM->SBUF load is a 128-partition DMA instruction so the HWDGE
    splits its descriptors across all 16 SDMA engines (sub-128-partition
    loads serialize on one engine at ~20 GB/s).
  - PE warmup matmuls + ACT table preload run during the input-DMA lead-in;
    MM2 of pair p is interleaved with MM1 of pair p+1 so the PE never
    idles.  Output is stored fp16 (cast back on host) to halve the final
    drain.
"""

import os
import numpy as np

import concourse.bacc as bacc
import concourse.mybir as mybir
from concourse import bass_utils
from concourse.tile import TileContext

F32 = mybir.dt.float32
F16 = mybir.dt.float16

S, B, HD = 256, 128, 42
J, CIN, HIDC = 21, 2, 64
NCORES = 8
SS = S // NCORES              # 32 seq positions per core
NTOK = SS * B                 # 4096 tokens per core
NHID = J * HIDC               # 1344 hidden channels per token
CH = 512                      # token chunk (= one PSUM bank of fp32)
NPAIR = NTOK // (2 * CH)      # 4 chunk pairs
NSPAN = 11                    # hidden k-tiles: 10 full (128) + one 64-wide
KAUG = HD + 1                 # 43 = 42 features + ones row
KT = 11                       # MM2 k-tiles over hidden
NWARM = 5                     # PE warmup matmuls during DMA lead-in
BN_EPS = 1e-5

_CACHE = {}
LAST_RESULT = None            # BassKernelResults of the most recent run (for test.py)


def _build_nc():
    nc = bacc.Bacc()
    x_ext = nc.declare_dram_parameter("x", [128, NPAIR, CH], F16, isOutput=False)
    m1_ext = nc.declare_dram_parameter("m1", [128, NSPAN, 128], F16, isOutput=False)
    m2_ext = nc.declare_dram_parameter("m2", [128, KT + 2, HD], F16, isOutput=False)
    out_ext = nc.declare_dram_parameter("out", [HD, NTOK], F16, isOutput=True)

    relu = mybir.ActivationFunctionType.Relu

    with TileContext(nc) as tc:
        with tc.tile_pool(name="const", bufs=1) as cpool, \
             tc.tile_pool(name="h1p", bufs=2) as h1pool, \
             tc.tile_pool(name="osb", bufs=2) as opool_sb, \
             tc.tile_pool(name="pm", bufs=3, space="PSUM") as mpool, \
             tc.tile_pool(name="po", bufs=2, space="PSUM") as popool:

            # --- input DMAs: every load covers all 128 partitions so its
            # descriptors fan out across the 16 SDMA engines.
            m1_sb = cpool.tile([128, NSPAN, 128], F16)
            nc.scalar.dma_start(out=m1_sb[:, 0:3], in_=m1_ext[:, 0:3])
            nc.scalar.dma_start(out=m1_sb[:, 3:NSPAN], in_=m1_ext[:, 3:NSPAN])
            m2_sb = cpool.tile([128, KT + 2, HD], F16)
            nc.scalar.dma_start(out=m2_sb, in_=m2_ext[:])

            x_sb = cpool.tile([128, NPAIR, CH], F16)
            for p in range(NPAIR):
                nc.sync.dma_start(out=x_sb[:, p], in_=x_ext[:, p])

            # --- PE warmup + ACT table preload, overlapped with input DMA.
            # memset on GpSimd: it is idle at start, so the warmup chain
            # begins ~1.5us earlier than via the Vector engine.
            wtile = cpool.tile([128, CH], F16)
            nc.gpsimd.memset(wtile, 0.0)
            nc.scalar.activation(wtile[0:1, 0:8], wtile[0:1, 8:16], relu)
            wps = popool.tile([128, CH], F32, tag="po")
            for _ in range(NWARM):
                nc.tensor.matmul(wps, wtile[:, 0:128], wtile, start=True, stop=True)

            # evac engine rotation: GpSimd cannot read PSUM on TRN2, so the
            # relu evacuation splits across Scalar (1.2 GHz) and Vector
            # (0.96 GHz) weighted by their throughput.
            def evac_scalar(dst, src):
                nc.scalar.activation(dst, src, relu)

            def evac_vector(dst, src):
                nc.vector.tensor_scalar_max(dst, src, 0.0)

            # per pair: 10 double evacs split 5/5; scalar takes the cheap
            # single (s=10) and the MM2 output copies
            rotation = [evac_scalar, evac_vector, evac_scalar, evac_vector,
                        evac_scalar, evac_vector, evac_scalar, evac_vector,
                        evac_scalar, evac_vector, evac_scalar]

            h1_of = {}

            def mm1_span(p, s, h1t):
                pt = mpool.tile([128, 1024], F32, tag="pm")
                if s < NSPAN - 1:
                    nc.tensor.matmul(
                        pt[:, 0:CH],
                        m1_sb[0:KAUG, s, :],
                        x_sb[0:KAUG, p],
                        start=True, stop=True, tile_position=(0, 0),
                    )
                    nc.tensor.matmul(
                        pt[:, CH:2 * CH],
                        m1_sb[64:64 + KAUG, s, :],
                        x_sb[64:64 + KAUG, p],
                        start=True, stop=True, tile_position=(64, 0),
                    )
                    rotation[s](h1t[:, s, :], pt[:, :])
                else:
                    # leftover 64 channels: chunk A -> psum partitions 0..63,
                    # chunk B -> 64..127, one 512-wide evacuation
                    nc.tensor.matmul(
                        pt[0:64, 0:CH],
                        m1_sb[0:KAUG, s, 0:64],
                        x_sb[0:KAUG, p],
                        start=True, stop=True, tile_position=(0, 0),
                    )
                    nc.tensor.matmul(
                        pt[64:128, 0:CH],
                        m1_sb[64:64 + KAUG, s, 0:64],
                        x_sb[64:64 + KAUG, p],
                        start=True, stop=True, tile_position=(64, 64),
                    )
                    rotation[s](h1t[:, s, 0:CH], pt[:, 0:CH])

            def mm2_steps(p, po, ts):
                h1t = h1_of[p]
                for t in ts:
                    st, sp = (t == 0), (t == KT)
                    if t < KT - 1:
                        nc.tensor.matmul(
                            po[0:HD, :], m2_sb[:, t, :],
                            h1t[:, t, 0:CH],
                            start=st, stop=sp, tile_position=(0, 0),
                        )
                        nc.tensor.matmul(
                            po[64:64 + HD, :], m2_sb[:, t, :],
                            h1t[:, t, CH:2 * CH],
                            start=st, stop=sp, tile_position=(0, 64),
                        )
                    elif t == KT - 1:
                        # leftover hidden k-tile: A at partitions 0..63,
                        # B at 64..127 (weights copy in slot 11 rows 64..127)
                        nc.tensor.matmul(
                            po[0:HD, :], m2_sb[0:64, t, :],
                            h1t[0:64, t, 0:CH],
                            start=st, stop=sp, tile_position=(0, 0),
                        )
                        nc.tensor.matmul(
                            po[64:64 + HD, :], m2_sb[64:128, KT, :],
                            h1t[64:128, t, 0:CH],
                            start=st, stop=sp, tile_position=(64, 64),
                        )
                    else:
                        # residual + bias shift, reading x directly
                        nc.tensor.matmul(
                            po[0:HD, :], m2_sb[0:KAUG, KT, :],
                            x_sb[0:KAUG, p],
                            start=st, stop=sp, tile_position=(0, 0),
                        )
                        nc.tensor.matmul(
                            po[64:64 + HD, :], m2_sb[64:64 + KAUG, KT + 1, :],
                            x_sb[64:64 + KAUG, p],
                            start=st, stop=sp, tile_position=(64, 64),
                        )

            def mm2_finish(p, po, last=False):
                osb = opool_sb.tile([106, CH], F16)
                if last:
                    # split the copy across both engines and generate the
                    # second DMA on the idle Act HWDGE to shorten the tail
                    nc.scalar.copy(osb[0:HD, :], po[0:HD, :])
                    nc.vector.tensor_copy(osb[64:106, :], po[64:106, :])
                    nc.sync.dma_start(out=out_ext[:, 1024 * p:1024 * p + CH],
                                      in_=osb[0:HD, :])
                    nc.scalar.dma_start(
                        out=out_ext[:, 1024 * p + CH:1024 * (p + 1)],
                        in_=osb[64:64 + HD, :])
                else:
                    nc.scalar.copy(osb, po[0:106, :])
                    nc.sync.dma_start(out=out_ext[:, 1024 * p:1024 * p + CH],
                                      in_=osb[0:HD, :])
                    nc.sync.dma_start(
                        out=out_ext[:, 1024 * p + CH:1024 * (p + 1)],
                        in_=osb[64:64 + HD, :])

            # --- pipeline: MM2 of pair p-1 interleaves with MM1 of pair p
            for p in range(NPAIR):
                h1t = h1pool.tile([128, NSPAN, 1024], F16)
                h1_of[p] = h1t
                if p == 0:
                    for s in range(NSPAN):
                        mm1_span(p, s, h1t)
                else:
                    prev = p - 1
                    po = popool.tile([128, CH], F32, tag="po")
                    for s in range(NSPAN):
                        mm1_span(p, s, h1t)
                        if s < 6:
                            mm2_steps(prev, po, (2 * s, 2 * s + 1))
                    mm2_finish(prev, po)
                    h1_of.pop(prev)
            # drain: MM2 of the last pair
            p = NPAIR - 1
            po = popool.tile([128, CH], F32, tag="po")
            mm2_steps(p, po, range(KT + 1))
            mm2_finish(p, po, last=True)

    nc.finalize()
    return nc


def _prep_weights(adj, W1, b1, W2, b2, g1, be1, m1, v1, g2, be2, m2, v2):
    adj = np.asarray(adj, np.float64)
    k1 = np.asarray(g1, np.float64) / np.sqrt(np.asarray(v1, np.float64) + BN_EPS)
    k2 = np.asarray(g2, np.float64) / np.sqrt(np.asarray(v2, np.float64) + BN_EPS)
    W1k = np.asarray(W1, np.float64) * k1[None, :]
    W2k = np.asarray(W2, np.float64) * k2[None, :]

    # M1[(j,c), (i,d)] = adj[i,j] * W1[c,d] * k1[d]; row j*2+c, col i*64+d
    M1 = np.einsum('ij,cd->jcid', adj, W1k).reshape(J * CIN, NHID)
    bias1 = np.tile((np.asarray(b1, np.float64) - np.asarray(m1, np.float64)) * k1
                    + np.asarray(be1, np.float64), J)
    M1a = np.concatenate([M1, bias1[None, :]], axis=0)          # (43, 1344)
    M1p = np.zeros((KAUG, NSPAN * 128), np.float32)
    M1p[:, :NHID] = M1a.astype(np.float32)
    m1_packed = np.zeros((128, NSPAN, 128), np.float16)
    for s in range(NSPAN):
        m1_packed[0:KAUG, s, :] = M1p[:, 128 * s:128 * (s + 1)].astype(np.float16)
        m1_packed[64:64 + KAUG, s, :] = m1_packed[0:KAUG, s, :]

    # M2[(j,d), (i,e)] = adj[i,j] * W2[d,e] * k2[e]; row j*64+d, col i*2+e
    M2 = np.einsum('ij,de->jdie', adj, W2k).reshape(NHID, HD)
    shift2 = np.tile((np.asarray(b2, np.float64) - np.asarray(m2, np.float64)) * k2
                     + np.asarray(be2, np.float64), J)
    m2_packed = np.zeros((128, KT + 2, HD), np.float32)
    M2p = np.zeros((KT * 128, HD), np.float32)
    M2p[:NHID] = M2.astype(np.float32)
    for t in range(KT):
        m2_packed[:, t, :] = M2p[t * 128:(t + 1) * 128, :]
    # slot KT (11): rows 0..42 = residual-A (identity + shift2 row), and
    # rows 64..127 = copy of hidden k-tile 10 for chunk B's PE row-tile.
    # slot KT+1 (12): rows 64..106 = residual-B.
    m2_packed[0:HD, KT, :] = np.eye(HD, dtype=np.float32)
    m2_packed[HD, KT, :] = shift2.astype(np.float32)
    m2_packed[64:128, KT, :] = m2_packed[0:64, KT - 1, :]
    m2_packed[64:64 + KAUG, KT + 1, :] = m2_packed[0:KAUG, KT, :]
    return m1_packed, m2_packed.astype(np.float16)


def kernel(x, adj, W1, b1, W2, b2, g1, be1, m1, v1, g2, be2, m2, v2):
    global LAST_RESULT
    x = np.asarray(x, np.float32)
    m1_packed, m2_packed = _prep_weights(adj, W1, b1, W2, b2,
                                         g1, be1, m1, v1, g2, be2, m2, v2)

    if "nc" not in _CACHE:
        _CACHE["nc"] = _build_nc()
    nc = _CACHE["nc"]

    in_maps = []
    for c in range(NCORES):
        xs = np.ascontiguousarray(
            x[c * SS:(c + 1) * SS].reshape(NTOK, HD).T).astype(np.float16)
        # chunk A of pair p -> partitions 0..42, chunk B -> 64..106
        xr = xs.reshape(HD, NPAIR, 2, CH)
        x_pk = np.zeros((128, NPAIR, CH), np.float16)
        x_pk[0:HD] = xr[:, :, 0, :]
        x_pk[HD] = 1.0
        x_pk[64:64 + HD] = xr[:, :, 1, :]
        x_pk[64 + HD] = 1.0
        in_maps.append({
            "x": x_pk,
            "m1": m1_packed,
            "m2": m2_packed,
        })

    trace = bool(int(os.environ.get("KERNEL_TRACE", "0")))
    res = bass_utils.run_bass_kernel_spmd(
        nc, in_maps, list(range(NCORES)), trace=trace,
    )
    LAST_RESULT = res

    out = np.empty((S, B, HD), np.float32)
    for c in range(NCORES):
        oc = res.results[c]["out"]                                # (42, NTOK)
        out[c * SS:(c + 1) * SS] = oc.astype(np.float32).T.reshape(SS, B, HD)
    return out
